# revision 29
# baseline (speedup 1.0000x reference)
"""Trainium2 Bass kernel for a quantized-conv BasicBlock.

  out = relu(BN2(conv3x3(relu(BN1(conv3x3(x, q(w1)))), q(w2))) + x)

Strategy: data-parallel over batch across 8 cores (4 images each).
BatchNorm statistics are global over the batch, so each core computes
per-channel partial sums (sum, sumsq) of the *unscaled integer* conv
output and a tiny [128,2] AllReduce produces the global stats.

Conv mapping: channels (128) live on SBUF partitions; a 3x3 pad=1 conv
is 9 PSUM-accumulated matmuls per 8-row output chunk (moving free dim
N=448), each reading a shifted window of a zero-padded [128,58,58]
image resident in SBUF.  LSQ-quantized weights are integer-valued
(w_q/alpha_s in {-4..3}) so they are exact in bf16; alpha_s is folded
into the BN affine on the host.  The whole datapath runs bf16
(activations, weights, output) with fp32 PSUM accumulation and fp32
stats — rel err ~4e-3, comfortably under the 2e-2 gate — halving
input/output DMA bytes and LDWEIGHTS time.

Perf notes vs the 243us baseline:
  * warmup AllReduce now has ZERO input dependencies (reads an
    ExternalInput directly) and is the first gpsimd instruction, so its
    mesh runs during the input DMA phase instead of queueing the CC
    core right before BN1's AllReduce (which cost ~17us of EQ_7 wait).
  * input DMAs ride only the two HWDGE rings (sync + scalar), image 0
    split across both rings first, so conv1 starts at ~3us not 18.7us.
  * sumsq stats moved from ScalarE (Square, which thrashed the
    activation table) to VectorE; ScalarE tables are preloaded with
    dummy ops so Sqrt/Relu switches are off the critical path.
  * stats collectives are issued from the sync engine (HWDGE
    completion latency) instead of gpsimd (SWDGE).
  * final fuse is per-quarter-image, relu on DVE via tensor_scalar
    (add,max), outputs stream out over both HWDGE rings as produced.
"""

import os
import numpy as np

N_CORES = 8
B, C, H, W = 32, 128, 56, 56
BL = B // N_CORES            # images per core
HP, WP = H + 2, W + 2        # padded image dims
PIX = H * W                  # 3136
PPIX = HP * WP               # 3364
RC = 8                       # output rows per PSUM chunk
NCHUNK = H // RC             # 7 chunks per image
NTOT = float(B * H * W)      # BN reduction size
BN_EPS = 1e-5
QN, QP = -4.0, 3.0           # 3-bit LSQ range
QROWS = 14                   # rows per output-fuse piece
NQ = H // QROWS              # 4 pieces per image

LAST_RESULTS = None          # BassKernelResults of the most recent run


def _quantize_int(w: np.ndarray, alpha: np.ndarray):
    """Replicate the reference LSQ forward math in fp32; return the
    integer-valued quantized weights (round(clip(w/alpha_s))) and alpha_s."""
    w = np.asarray(w, dtype=np.float32)
    alpha = np.float32(np.asarray(alpha, dtype=np.float32).reshape(-1)[0])
    g = np.float32(1.0) / np.sqrt(np.float32(w.size * 3.0))
    ag = np.float32(alpha * g)
    alpha_s = np.float32(ag + np.float32(alpha - ag))
    with np.errstate(divide="ignore", invalid="ignore"):
        wc = np.clip((w / alpha_s).astype(np.float32), np.float32(QN), np.float32(QP))
    wq = np.rint(wc).astype(np.float32)
    return wq, alpha_s


def _build_program(as1: float, as2: float):
    import concourse.bacc as bacc
    import concourse.tile as tile
    import concourse.mybir as mybir

    f32 = mybir.dt.float32
    bf16 = mybir.dt.bfloat16
    AF = mybir.ActivationFunctionType
    ALU = mybir.AluOpType
    AX = mybir.AxisListType

    nc = bacc.Bacc("TRN2", target_bir_lowering=False, debug=False,
                   num_devices=N_CORES)

    xp_d = nc.dram_tensor("xp", [BL, C, HP, WP], bf16, kind="ExternalInput")
    w1_d = nc.dram_tensor("w1t", [C, 9, C], bf16, kind="ExternalInput")
    w2_d = nc.dram_tensor("w2t", [C, 9, C], bf16, kind="ExternalInput")
    id_d = nc.dram_tensor("ident", [C, C], bf16, kind="ExternalInput")
    ga1_d = nc.dram_tensor("ga1", [C, 1], f32, kind="ExternalInput")
    be1_d = nc.dram_tensor("be1", [C, 1], f32, kind="ExternalInput")
    ga2_d = nc.dram_tensor("ga2", [C, 1], f32, kind="ExternalInput")
    be2_d = nc.dram_tensor("be2", [C, 1], f32, kind="ExternalInput")
    y_d = nc.dram_tensor("y", [BL, C, PIX], bf16, kind="ExternalOutput")

    groups = [list(range(N_CORES))]

    with tile.TileContext(nc) as tc:
        with (
            tc.tile_pool(name="persist", bufs=1) as persist,
            tc.tile_pool(name="xp_p", bufs=BL) as xp_p,
            tc.tile_pool(name="a1_p", bufs=BL) as a1_p,
            tc.tile_pool(name="o2_p", bufs=BL) as o2_p,
            tc.tile_pool(name="psum", bufs=8, space="PSUM") as psum_p,
            tc.tile_pool(name="dram", bufs=1, space="DRAM") as dram_p,
        ):
            # ---- warmup collective: zero input deps, first thing on the
            # CC queue.  Its mesh absorbs rank start skew + first-collective
            # staging cost during the input-DMA/conv1 phase, so the BN1
            # AllReduce hits a warm, idle CC core.
            # The collective input is an UNINITIALIZED DRAM tile on purpose:
            # the warmup's result is never consumed, and having zero input
            # dependencies lets the trigger fire within ~1us of NEFF start,
            # so the CC path's one-time ~53us setup completes during conv1.
            wci = dram_p.tile([C, 1], f32, tag="wci", name="wci")
            wco = dram_p.tile([C, 1], f32, tag="wco", name="wco")
            nc.gpsimd.collective_compute(
                "AllReduce", ALU.add, replica_groups=groups,
                ins=[wci.opt()], outs=[wco.opt()],
            )

            # ---- weights / inputs on the two HWDGE rings, priority order.
            # ring A = sync, ring B = scalar; image b split across both.
            w1_t = persist.tile([C, 9, C], bf16, tag="w1", name="w1")
            w2_t = persist.tile([C, 9, C], bf16, tag="w2", name="w2")
            id_t = persist.tile([C, C], bf16, tag="id", name="id")
            xp_t = []
            for b in range(BL):
                xp_t.append(xp_p.tile([C, HP, WP], bf16, tag="xp", name=f"xp{b}"))
            # image 0 arrives in three pieces so conv1's first chunks can
            # start as soon as rows 0..10 + w1 land (~10us instead of ~14)
            nc.sync.dma_start(xp_t[0][:, :11, :], xp_d.ap()[0][:, :11, :])
            nc.scalar.dma_start(w1_t[:], w1_d.ap())
            nc.sync.dma_start(xp_t[0][:, 11:30, :], xp_d.ap()[0][:, 11:30, :])
            nc.scalar.dma_start(xp_t[0][:, 30:, :], xp_d.ap()[0][:, 30:, :])
            HS = 29
            for b in range(1, BL):
                nc.sync.dma_start(xp_t[b][:, :HS, :], xp_d.ap()[b][:, :HS, :])
                nc.scalar.dma_start(xp_t[b][:, HS:, :], xp_d.ap()[b][:, HS:, :])

            ga1 = persist.tile([C, 1], f32, tag="ga1", name="ga1")
            be1 = persist.tile([C, 1], f32, tag="be1", name="be1")
            ga2 = persist.tile([C, 1], f32, tag="ga2", name="ga2")
            be2 = persist.tile([C, 1], f32, tag="be2", name="be2")
            nc.scalar.dma_start(ga1[:], ga1_d.ap())
            nc.scalar.dma_start(be1[:], be1_d.ap())
            nc.scalar.dma_start(ga2[:], ga2_d.ap())
            nc.scalar.dma_start(be2[:], be2_d.ap())
            nc.scalar.dma_start(w2_t[:], w2_d.ap())
            nc.scalar.dma_start(id_t[:], id_d.ap())

            dum = persist.tile([C, 1], f32, tag="dum", name="dum")

            # ---- per-image persistent buffers ----------------------------
            a1_t, o2_t = [], []
            for b in range(BL):
                at = a1_p.tile([C, HP, WP], bf16, tag="a1", name=f"a1_{b}")
                # zero the 1-pixel border once; interior is fully overwritten
                nc.vector.memset(at[:, 0, :], 0.0)
                nc.vector.memset(at[:, HP - 1, :], 0.0)
                nc.vector.memset(at[:, 1:HP - 1, 0], 0.0)
                nc.vector.memset(at[:, 1:HP - 1, WP - 1], 0.0)
                a1_t.append(at)
                o2_t.append(o2_p.tile([C, H, W], bf16, tag="o2", name=f"o2_{b}"))

            # partial-stat columns: one col per (image, chunk)
            s1a = persist.tile([C, BL * NCHUNK], f32, tag="s1a", name="s1a")
            s2a = persist.tile([C, BL * NCHUNK], f32, tag="s2a", name="s2a")
            s1b = persist.tile([C, BL * NCHUNK], f32, tag="s1b", name="s1b")
            s2b = persist.tile([C, BL * NCHUNK], f32, tag="s2b", name="s2b")
            # write target for the squares pass (accum_out needs an out AP)
            scr = persist.tile([C, RC, W], bf16, tag="scr", name="scr")

            def conv(src_tiles, w_t, dst, s1cols, s2cols, pre_image=None):
                """3x3 conv of all images; dst(b, chunk) -> out AP with free
                dims [RC, W].  Accumulates per-chunk stats (sum on DVE, sumsq
                on ScalarE).  pre_image(b) emits per-image preamble ops just
                before image b's chunks (keeps ScalarE's FIFO interleaved)."""
                for b in range(BL):
                    if pre_image is not None:
                        pre_image(b)
                    src = src_tiles[b]
                    for ci in range(NCHUNK):
                        r0 = ci * RC
                        ps = psum_p.tile([C, RC, W], f32, tag="ps",
                                         name=f"ps_{b}_{ci}")
                        for t in range(9):
                            kh, kw = t // 3, t % 3
                            rhs = src[:, r0 + kh:r0 + kh + RC, kw:kw + W]
                            nc.tensor.matmul(
                                ps[:], w_t[:, t, :], rhs,
                                start=(t == 0), stop=(t == 8),
                            )
                        idx = b * NCHUNK + ci
                        nc.vector.tensor_scalar(
                            out=dst(b, ci), in0=ps[:],
                            scalar1=0.0, scalar2=0.0, op0=ALU.add, op1=ALU.add,
                            accum_out=s1cols[:, idx:idx + 1],
                        )
                        nc.scalar.activation(
                            scr[:], ps[:], AF.Square,
                            accum_out=s2cols[:, idx:idx + 1],
                        )

            def bn_params(s1cols, s2cols, gam, bet, alpha_s, pref):
                """Reduce partials, AllReduce across cores, produce per-channel
                affine (a, b) implementing BN on the unscaled conv output."""
                # the two final reduces run on different engines in parallel
                # (ScalarE Copy+accum does a sum reduce without a table load)
                cc_in = persist.tile([C, 2], f32, tag=pref + "ci", name=pref + "ci")
                nc.vector.tensor_reduce(cc_in[:, 0:1], s1cols[:], axis=AX.X,
                                        op=ALU.add)
                nc.scalar.activation(scr[:, 0, :BL * NCHUNK], s2cols[:],
                                     AF.Copy, accum_out=cc_in[:, 1:2])
                d_in = dram_p.tile([C, 2], f32, tag=pref + "di", name=pref + "di")
                d_out = dram_p.tile([C, 2], f32, tag=pref + "do", name=pref + "do")
                nc.sync.dma_start(d_in[:], cc_in[:])
                nc.gpsimd.collective_compute(
                    "AllReduce", ALU.add, replica_groups=groups,
                    ins=[d_in.opt()], outs=[d_out.opt()],
                )
                # gpsimd issues the readback: it is already blocked on the
                # collective, so its next instruction dispatches the moment
                # the mesh completes (no cross-engine semaphore hop)
                gst = persist.tile([C, 2], f32, tag=pref + "gs", name=pref + "gs")
                nc.gpsimd.dma_start(gst[:], d_out[:])

                mun = persist.tile([C, 1], f32, tag=pref + "mu", name=pref + "mu")
                e2 = persist.tile([C, 1], f32, tag=pref + "e2", name=pref + "e2")
                va = persist.tile([C, 1], f32, tag=pref + "va", name=pref + "va")
                rs = persist.tile([C, 1], f32, tag=pref + "rs", name=pref + "rs")
                a_ = persist.tile([C, 1], f32, tag=pref + "a", name=pref + "a")
                b_ = persist.tile([C, 1], f32, tag=pref + "b", name=pref + "b")
                inv_n = float(1.0 / NTOT)
                # mun = -mean_int ; e2 = E[z^2]_int
                nc.vector.tensor_scalar_mul(mun[:], gst[:, 0:1], -inv_n)
                nc.vector.tensor_scalar_mul(e2[:], gst[:, 1:2], inv_n)
                # va = mun^2 - e2 = -var_int ; then var_true+eps = -as^2*va+eps
                nc.vector.scalar_tensor_tensor(
                    out=va[:], in0=mun[:], scalar=mun[:], in1=e2[:],
                    op0=ALU.mult, op1=ALU.subtract)
                nc.vector.tensor_scalar(
                    out=va[:], in0=va[:],
                    scalar1=float(-(alpha_s ** 2)), scalar2=BN_EPS,
                    op0=ALU.mult, op1=ALU.add)
                nc.vector.reciprocal(rs[:], va[:])
                nc.scalar.activation(rs[:], rs[:], AF.Sqrt)
                # a = gamma * alpha_s * rstd ; b = beta + mun * a
                # (gam already folded with alpha_s on host: gam = gamma*alpha_s)
                nc.vector.tensor_mul(a_[:], gam[:], rs[:])
                nc.vector.scalar_tensor_tensor(
                    out=b_[:], in0=mun[:], scalar=a_[:], in1=bet[:],
                    op0=ALU.mult, op1=ALU.add)
                return a_, b_

            # ================= conv1 =====================================
            conv(xp_t, w1_t,
                 lambda b, ci: a1_t[b][:, 1 + ci * RC:1 + ci * RC + RC, 1:1 + W],
                 s1a, s2a)

            # preload the Sqrt table right after conv1's Square ops drain,
            # during the AllReduce wait
            nc.scalar.activation(dum[:], ga1[:], AF.Sqrt)

            a1c, b1c = bn_params(s1a, s2a, ga1, be1, as1, "p")

            # preload the Relu table in parallel with the BN1 param chain
            nc.scalar.activation(dum[:], ga1[:], AF.Relu)

            # BN1 + relu in place on the act1 interior, emitted per image
            # right before that image's conv2 chunks (so ScalarE's FIFO does
            # not stall conv2's PSUM-releasing Square ops); first slice small
            # so conv2 can start early.
            def bn1_apply(b):
                for (lo, hi) in ((0, 11), (11, 33), (33, 56)):
                    iv = a1_t[b][:, 1 + lo:1 + hi, 1:1 + W]
                    nc.scalar.activation(iv, iv, AF.Relu,
                                         bias=b1c[:], scale=a1c[:])

            # ================= conv2 =====================================
            conv(a1_t, w2_t,
                 lambda b, ci: o2_t[b][:, ci * RC:ci * RC + RC, :],
                 s1b, s2b, pre_image=bn1_apply)

            # switch ScalarE back to Sqrt while conv2 runs
            nc.scalar.activation(dum[:], ga1[:], AF.Sqrt)

            # final: y = relu(a2*z2 + b2 + x), computed on the (otherwise
            # idle) tensor engine: per conv-chunk, PSUM = I @ x + diag(a2)
            # @ z2, then one 1-input drain op relu(ps + b2) alternating
            # DVE/ScalarE, then DMA out on alternating HWDGE rings.  This
            # avoids the DVE 2-tensor-input op that runs 3x slower than
            # 1-input ops.  The I @ x matmuls need no BN2 params, so the
            # first 8 banks' worth run DURING the AllReduce wait (also
            # keeping the PE's power state warm through the barrier).
            NFC = BL * NCHUNK
            fchunk = []
            for k in range(NFC):
                b, ci = k // NCHUNK, k % NCHUNK
                fchunk.append((b, ci * RC))
            fps = {}

            def prestage(k):
                b, r0 = fchunk[k]
                fps[k] = psum_p.tile([C, RC, W], f32, tag="ps",
                                     name=f"fps_{k}")
                nc.tensor.matmul(fps[k][:], id_t[:],
                                 xp_t[b][:, 1 + r0:1 + r0 + RC, 1:1 + W],
                                 start=True, stop=False)

            for k in range(8):
                prestage(k)

            a2c, b2c = bn_params(s1b, s2b, ga2, be2, as2, "q")

            # ScalarE builds diag(a2) (Copy needs no table load) while DVE
            # finishes the b2 half of the param chain, then preloads Relu
            dgw = persist.tile([C, C], bf16, tag="dgw", name="dgw")
            nc.scalar.activation(dgw[:], id_t[:], AF.Copy, scale=a2c[:])
            nc.scalar.activation(dum[:], ga1[:], AF.Relu)
            for k in range(NFC):
                b, r0 = fchunk[k]
                ps = fps.pop(k)
                nc.tensor.matmul(ps[:], dgw[:], o2_t[b][:, r0:r0 + RC, :],
                                 start=False, stop=True)
                u = o2_t[b][:, r0:r0 + RC, :]
                if k % 3 == 2:
                    # ScalarE's PSUM-source Relu is ~1.6x slower than DVE's
                    # tensor_scalar, so it only takes every third drain
                    nc.scalar.activation(u, ps[:], AF.Relu, bias=b2c[:],
                                         scale=1.0)
                else:
                    nc.vector.tensor_scalar(
                        out=u, in0=ps[:], scalar1=b2c[:], scalar2=0.0,
                        op0=ALU.add, op1=ALU.max)
                eng = nc.sync if k % 2 == 0 else nc.scalar
                eng.dma_start(y_d.ap()[b][:, r0 * W:(r0 + RC) * W], u)
                if k + 8 < NFC:
                    prestage(k + 8)

    nc.compile()
    return nc


def _prep_inputs(x, w1, alpha1, gamma1, beta1, w2, alpha2, gamma2, beta2):
    import ml_dtypes
    bf = ml_dtypes.bfloat16
    x = np.asarray(x, dtype=np.float32)
    wq1, as1 = _quantize_int(np.asarray(w1), np.asarray(alpha1))
    wq2, as2 = _quantize_int(np.asarray(w2), np.asarray(alpha2))

    # [cout, cin, kh, kw] -> [cin, tap, cout] so lhsT slices are [K=cin, M=cout]
    w1t = np.ascontiguousarray(
        wq1.reshape(C, C, 9).transpose(1, 2, 0)).astype(bf)
    w2t = np.ascontiguousarray(
        wq2.reshape(C, C, 9).transpose(1, 2, 0)).astype(bf)

    ga1 = (np.asarray(gamma1, np.float32) * as1).reshape(C, 1)
    ga2 = (np.asarray(gamma2, np.float32) * as2).reshape(C, 1)
    be1 = np.asarray(beta1, np.float32).reshape(C, 1).copy()
    be2 = np.asarray(beta2, np.float32).reshape(C, 1).copy()
    ident = np.eye(C, dtype=bf)

    xpad = np.zeros((B, C, HP, WP), dtype=bf)
    xpad[:, :, 1:1 + H, 1:1 + W] = x.astype(bf)

    in_maps = []
    for c in range(N_CORES):
        shard = xpad[c * BL:(c + 1) * BL]
        in_maps.append({
            "xp": np.ascontiguousarray(shard),
            "w1t": w1t, "w2t": w2t, "ident": ident,
            "ga1": ga1, "be1": be1, "ga2": ga2, "be2": be2,
        })
    return in_maps, float(as1), float(as2)


def kernel(**inputs) -> np.ndarray:
    global LAST_RESULTS
    from concourse.bass_utils import run_bass_kernel_spmd

    in_maps, as1, as2 = _prep_inputs(**inputs)
    nc = _build_program(as1, as2)

    trace = bool(int(os.environ.get("KERNEL_TRACE", "0")))
    res = run_bass_kernel_spmd(
        nc, in_maps, list(range(N_CORES)),
        trace=trace,
    )
    LAST_RESULTS = res
    out = np.stack([np.asarray(res.results[c]["y"]) for c in range(N_CORES)])
    return np.ascontiguousarray(
        out.reshape(B, C, H, W)).astype(np.float32)


# revision 30
# speedup vs baseline: 1.0198x; 1.0198x over previous
"""Trainium2 Bass kernel for a quantized-conv BasicBlock.

  out = relu(BN2(conv3x3(relu(BN1(conv3x3(x, q(w1)))), q(w2))) + x)

Strategy: data-parallel over batch across 8 cores (4 images each).
BatchNorm statistics are global over the batch, so each core computes
per-channel partial sums (sum, sumsq) of the *unscaled integer* conv
output and a tiny [128,2] AllReduce produces the global stats.

Conv mapping: channels (128) live on SBUF partitions; a 3x3 pad=1 conv
is 9 PSUM-accumulated matmuls per 8-row output chunk (moving free dim
N=448), each reading a shifted window of a zero-padded [128,58,58]
image resident in SBUF.  LSQ-quantized weights are integer-valued
(w_q/alpha_s in {-4..3}) so they are exact in bf16; alpha_s is folded
into the BN affine on the host.  The whole datapath runs bf16
(activations, weights, output) with fp32 PSUM accumulation and fp32
stats — rel err ~4e-3, comfortably under the 2e-2 gate — halving
input/output DMA bytes and LDWEIGHTS time.

Perf notes vs the 243us baseline:
  * warmup AllReduce now has ZERO input dependencies (reads an
    ExternalInput directly) and is the first gpsimd instruction, so its
    mesh runs during the input DMA phase instead of queueing the CC
    core right before BN1's AllReduce (which cost ~17us of EQ_7 wait).
  * input DMAs ride only the two HWDGE rings (sync + scalar), image 0
    split across both rings first, so conv1 starts at ~3us not 18.7us.
  * sumsq stats moved from ScalarE (Square, which thrashed the
    activation table) to VectorE; ScalarE tables are preloaded with
    dummy ops so Sqrt/Relu switches are off the critical path.
  * stats collectives are issued from the sync engine (HWDGE
    completion latency) instead of gpsimd (SWDGE).
  * final fuse is per-quarter-image, relu on DVE via tensor_scalar
    (add,max), outputs stream out over both HWDGE rings as produced.
"""

import os
import numpy as np

N_CORES = 8
B, C, H, W = 32, 128, 56, 56
BL = B // N_CORES            # images per core
HP, WP = H + 2, W + 2        # padded image dims
PIX = H * W                  # 3136
PPIX = HP * WP               # 3364
RC = 8                       # output rows per PSUM chunk
NCHUNK = H // RC             # 7 chunks per image
NTOT = float(B * H * W)      # BN reduction size
BN_EPS = 1e-5
QN, QP = -4.0, 3.0           # 3-bit LSQ range
QROWS = 14                   # rows per output-fuse piece
NQ = H // QROWS              # 4 pieces per image

LAST_RESULTS = None          # BassKernelResults of the most recent run


def _quantize_int(w: np.ndarray, alpha: np.ndarray):
    """Replicate the reference LSQ forward math in fp32; return the
    integer-valued quantized weights (round(clip(w/alpha_s))) and alpha_s."""
    w = np.asarray(w, dtype=np.float32)
    alpha = np.float32(np.asarray(alpha, dtype=np.float32).reshape(-1)[0])
    g = np.float32(1.0) / np.sqrt(np.float32(w.size * 3.0))
    ag = np.float32(alpha * g)
    alpha_s = np.float32(ag + np.float32(alpha - ag))
    with np.errstate(divide="ignore", invalid="ignore"):
        wc = np.clip((w / alpha_s).astype(np.float32), np.float32(QN), np.float32(QP))
    wq = np.rint(wc).astype(np.float32)
    return wq, alpha_s


def _build_program(as1: float, as2: float):
    import concourse.bacc as bacc
    import concourse.tile as tile
    import concourse.mybir as mybir

    f32 = mybir.dt.float32
    bf16 = mybir.dt.bfloat16
    AF = mybir.ActivationFunctionType
    ALU = mybir.AluOpType
    AX = mybir.AxisListType

    nc = bacc.Bacc("TRN2", target_bir_lowering=False, debug=False,
                   num_devices=N_CORES)

    xp_d = nc.dram_tensor("xp", [BL, C, HP, WP], bf16, kind="ExternalInput")
    w1_d = nc.dram_tensor("w1t", [C, 9, C], bf16, kind="ExternalInput")
    w2_d = nc.dram_tensor("w2t", [C, 9, C], bf16, kind="ExternalInput")
    id_d = nc.dram_tensor("ident", [C, C], bf16, kind="ExternalInput")
    ga1_d = nc.dram_tensor("ga1", [C, 1], f32, kind="ExternalInput")
    be1_d = nc.dram_tensor("be1", [C, 1], f32, kind="ExternalInput")
    ga2_d = nc.dram_tensor("ga2", [C, 1], f32, kind="ExternalInput")
    be2_d = nc.dram_tensor("be2", [C, 1], f32, kind="ExternalInput")
    y_d = nc.dram_tensor("y", [BL, C, PIX], bf16, kind="ExternalOutput")

    groups = [list(range(N_CORES))]

    with tile.TileContext(nc) as tc:
        with (
            tc.tile_pool(name="persist", bufs=1) as persist,
            tc.tile_pool(name="xp_p", bufs=BL) as xp_p,
            tc.tile_pool(name="a1_p", bufs=BL) as a1_p,
            tc.tile_pool(name="o2_p", bufs=BL) as o2_p,
            tc.tile_pool(name="psum", bufs=8, space="PSUM") as psum_p,
            tc.tile_pool(name="dram", bufs=1, space="DRAM") as dram_p,
        ):
            # ---- warmup collective: zero input deps, first thing on the
            # CC queue.  Its mesh absorbs rank start skew + first-collective
            # staging cost during the input-DMA/conv1 phase, so the BN1
            # AllReduce hits a warm, idle CC core.
            # The collective input is an UNINITIALIZED DRAM tile on purpose:
            # the warmup's result is never consumed, and having zero input
            # dependencies lets the trigger fire within ~1us of NEFF start,
            # so the CC path's one-time ~53us setup completes during conv1.
            wci = dram_p.tile([C, 1], f32, tag="wci", name="wci")
            wco = dram_p.tile([C, 1], f32, tag="wco", name="wco")
            nc.gpsimd.collective_compute(
                "AllReduce", ALU.add, replica_groups=groups,
                ins=[wci.opt()], outs=[wco.opt()],
            )

            # ---- weights / inputs on the two HWDGE rings, priority order.
            # ring A = sync, ring B = scalar; image b split across both.
            w1_t = persist.tile([C, 9, C], bf16, tag="w1", name="w1")
            w2_t = persist.tile([C, 9, C], bf16, tag="w2", name="w2")
            id_t = persist.tile([C, C], bf16, tag="id", name="id")
            xp_t = []
            for b in range(BL):
                xp_t.append(xp_p.tile([C, HP, WP], bf16, tag="xp", name=f"xp{b}"))
            # image 0 arrives in three pieces so conv1's first chunks can
            # start as soon as rows 0..10 + w1 land (~10us instead of ~14)
            nc.sync.dma_start(xp_t[0][:, :11, :], xp_d.ap()[0][:, :11, :])
            nc.scalar.dma_start(w1_t[:], w1_d.ap())
            nc.sync.dma_start(xp_t[0][:, 11:30, :], xp_d.ap()[0][:, 11:30, :])
            nc.scalar.dma_start(xp_t[0][:, 30:, :], xp_d.ap()[0][:, 30:, :])
            HS = 29
            for b in range(1, BL):
                nc.sync.dma_start(xp_t[b][:, :HS, :], xp_d.ap()[b][:, :HS, :])
                nc.scalar.dma_start(xp_t[b][:, HS:, :], xp_d.ap()[b][:, HS:, :])

            ga1 = persist.tile([C, 1], f32, tag="ga1", name="ga1")
            be1 = persist.tile([C, 1], f32, tag="be1", name="be1")
            ga2 = persist.tile([C, 1], f32, tag="ga2", name="ga2")
            be2 = persist.tile([C, 1], f32, tag="be2", name="be2")
            nc.scalar.dma_start(ga1[:], ga1_d.ap())
            nc.scalar.dma_start(be1[:], be1_d.ap())
            nc.scalar.dma_start(ga2[:], ga2_d.ap())
            nc.scalar.dma_start(be2[:], be2_d.ap())
            nc.scalar.dma_start(w2_t[:], w2_d.ap())
            nc.scalar.dma_start(id_t[:], id_d.ap())

            dum = persist.tile([C, 1], f32, tag="dum", name="dum")

            # ---- per-image persistent buffers ----------------------------
            a1_t, o2_t = [], []
            for b in range(BL):
                at = a1_p.tile([C, HP, WP], bf16, tag="a1", name=f"a1_{b}")
                # zero the 1-pixel border once; interior is fully overwritten
                nc.vector.memset(at[:, 0, :], 0.0)
                nc.vector.memset(at[:, HP - 1, :], 0.0)
                nc.vector.memset(at[:, 1:HP - 1, 0], 0.0)
                nc.vector.memset(at[:, 1:HP - 1, WP - 1], 0.0)
                a1_t.append(at)
                o2_t.append(o2_p.tile([C, H, W], bf16, tag="o2", name=f"o2_{b}"))

            # partial-stat columns: one col per (image, chunk)
            s1a = persist.tile([C, BL * NCHUNK], f32, tag="s1a", name="s1a")
            s2a = persist.tile([C, BL * NCHUNK], f32, tag="s2a", name="s2a")
            s1b = persist.tile([C, BL * NCHUNK], f32, tag="s1b", name="s1b")
            s2b = persist.tile([C, BL * NCHUNK], f32, tag="s2b", name="s2b")
            # write target for the squares pass (accum_out needs an out AP)
            scr = persist.tile([C, RC, W], bf16, tag="scr", name="scr")

            def conv(src_tiles, w_t, dst, s1cols, s2cols, pre_image=None):
                """3x3 conv of all images; dst(b, chunk) -> out AP with free
                dims [RC, W].  Accumulates per-chunk stats (sum on DVE, sumsq
                on ScalarE).  pre_image(b) emits per-image preamble ops just
                before image b's chunks (keeps ScalarE's FIFO interleaved)."""
                for b in range(BL):
                    if pre_image is not None:
                        pre_image(b)
                    src = src_tiles[b]
                    for ci in range(NCHUNK):
                        r0 = ci * RC
                        ps = psum_p.tile([C, RC, W], f32, tag="ps",
                                         name=f"ps_{b}_{ci}")
                        for t in range(9):
                            kh, kw = t // 3, t % 3
                            rhs = src[:, r0 + kh:r0 + kh + RC, kw:kw + W]
                            nc.tensor.matmul(
                                ps[:], w_t[:, t, :], rhs,
                                start=(t == 0), stop=(t == 8),
                            )
                        idx = b * NCHUNK + ci
                        nc.vector.tensor_scalar(
                            out=dst(b, ci), in0=ps[:],
                            scalar1=0.0, scalar2=0.0, op0=ALU.add, op1=ALU.add,
                            accum_out=s1cols[:, idx:idx + 1],
                        )
                        nc.scalar.activation(
                            scr[:], ps[:], AF.Square,
                            accum_out=s2cols[:, idx:idx + 1],
                        )

            def bn_params(s1cols, s2cols, gam, bet, alpha_s, pref):
                """Reduce partials, AllReduce across cores, produce per-channel
                affine (a, b) implementing BN on the unscaled conv output."""
                # the two final reduces run on different engines in parallel
                # (ScalarE Copy+accum does a sum reduce without a table load)
                cc_in = persist.tile([C, 2], f32, tag=pref + "ci", name=pref + "ci")
                nc.vector.tensor_reduce(cc_in[:, 0:1], s1cols[:], axis=AX.X,
                                        op=ALU.add)
                nc.scalar.activation(scr[:, 0, :BL * NCHUNK], s2cols[:],
                                     AF.Copy, accum_out=cc_in[:, 1:2])
                d_in = dram_p.tile([C, 2], f32, tag=pref + "di", name=pref + "di")
                d_out = dram_p.tile([C, 2], f32, tag=pref + "do", name=pref + "do")
                nc.sync.dma_start(d_in[:], cc_in[:])
                nc.gpsimd.collective_compute(
                    "AllReduce", ALU.add, replica_groups=groups,
                    ins=[d_in.opt()], outs=[d_out.opt()],
                )
                # gpsimd issues the readback: it is already blocked on the
                # collective, so its next instruction dispatches the moment
                # the mesh completes (no cross-engine semaphore hop)
                gst = persist.tile([C, 2], f32, tag=pref + "gs", name=pref + "gs")
                nc.gpsimd.dma_start(gst[:], d_out[:])

                mun = persist.tile([C, 1], f32, tag=pref + "mu", name=pref + "mu")
                e2 = persist.tile([C, 1], f32, tag=pref + "e2", name=pref + "e2")
                va = persist.tile([C, 1], f32, tag=pref + "va", name=pref + "va")
                rs = persist.tile([C, 1], f32, tag=pref + "rs", name=pref + "rs")
                a_ = persist.tile([C, 1], f32, tag=pref + "a", name=pref + "a")
                b_ = persist.tile([C, 1], f32, tag=pref + "b", name=pref + "b")
                inv_n = float(1.0 / NTOT)
                # mun = -mean_int ; e2 = E[z^2]_int
                nc.vector.tensor_scalar_mul(mun[:], gst[:, 0:1], -inv_n)
                nc.vector.tensor_scalar_mul(e2[:], gst[:, 1:2], inv_n)
                # va = mun^2 - e2 = -var_int ; then var_true+eps = -as^2*va+eps
                nc.vector.scalar_tensor_tensor(
                    out=va[:], in0=mun[:], scalar=mun[:], in1=e2[:],
                    op0=ALU.mult, op1=ALU.subtract)
                nc.vector.tensor_scalar(
                    out=va[:], in0=va[:],
                    scalar1=float(-(alpha_s ** 2)), scalar2=BN_EPS,
                    op0=ALU.mult, op1=ALU.add)
                nc.vector.reciprocal(rs[:], va[:])
                nc.scalar.activation(rs[:], rs[:], AF.Sqrt)
                # a = gamma * alpha_s * rstd ; b = beta + mun * a
                # (gam already folded with alpha_s on host: gam = gamma*alpha_s)
                nc.vector.tensor_mul(a_[:], gam[:], rs[:])
                nc.vector.scalar_tensor_tensor(
                    out=b_[:], in0=mun[:], scalar=a_[:], in1=bet[:],
                    op0=ALU.mult, op1=ALU.add)
                return a_, b_

            # ================= conv1 =====================================
            conv(xp_t, w1_t,
                 lambda b, ci: a1_t[b][:, 1 + ci * RC:1 + ci * RC + RC, 1:1 + W],
                 s1a, s2a)

            # preload the Sqrt table right after conv1's Square ops drain,
            # during the AllReduce wait
            nc.scalar.activation(dum[:], ga1[:], AF.Sqrt)

            a1c, b1c = bn_params(s1a, s2a, ga1, be1, as1, "p")

            # preload the Relu table in parallel with the BN1 param chain
            nc.scalar.activation(dum[:], ga1[:], AF.Relu)

            # BN1 + relu in place on the act1 interior, emitted per image
            # right before that image's conv2 chunks (so ScalarE's FIFO does
            # not stall conv2's PSUM-releasing Square ops); first slice small
            # so conv2 can start early.
            def bn1_apply(b):
                for (lo, hi) in ((0, 11), (11, 33), (33, 56)):
                    iv = a1_t[b][:, 1 + lo:1 + hi, 1:1 + W]
                    nc.scalar.activation(iv, iv, AF.Relu,
                                         bias=b1c[:], scale=a1c[:])

            # ================= conv2 =====================================
            conv(a1_t, w2_t,
                 lambda b, ci: o2_t[b][:, ci * RC:ci * RC + RC, :],
                 s1b, s2b, pre_image=bn1_apply)

            # switch ScalarE back to Sqrt while conv2 runs
            nc.scalar.activation(dum[:], ga1[:], AF.Sqrt)

            # final: y = relu(a2*z2 + b2 + x), computed on the (otherwise
            # idle) tensor engine: per conv-chunk, PSUM = I @ x + diag(a2)
            # @ z2, then one 1-input drain op relu(ps + b2) alternating
            # DVE/ScalarE, then DMA out on alternating HWDGE rings.  This
            # avoids the DVE 2-tensor-input op that runs 3x slower than
            # 1-input ops.  The I @ x matmuls need no BN2 params, so the
            # first 8 banks' worth run DURING the AllReduce wait (also
            # keeping the PE's power state warm through the barrier).
            NFC = BL * NCHUNK
            fchunk = []
            for k in range(NFC):
                b, ci = k // NCHUNK, k % NCHUNK
                fchunk.append((b, ci * RC))
            fps = {}

            def prestage(k):
                b, r0 = fchunk[k]
                fps[k] = psum_p.tile([C, RC, W], f32, tag="ps",
                                     name=f"fps_{k}")
                nc.tensor.matmul(fps[k][:], id_t[:],
                                 xp_t[b][:, 1 + r0:1 + r0 + RC, 1:1 + W],
                                 start=True, stop=False)

            for k in range(8):
                prestage(k)

            a2c, b2c = bn_params(s1b, s2b, ga2, be2, as2, "q")

            # ScalarE builds diag(a2) (Copy needs no table load) while DVE
            # finishes the b2 half of the param chain, then preloads Relu
            dgw = persist.tile([C, C], bf16, tag="dgw", name="dgw")
            nc.scalar.activation(dgw[:], id_t[:], AF.Copy, scale=a2c[:])
            nc.scalar.activation(dum[:], ga1[:], AF.Relu)
            for k in range(NFC):
                b, r0 = fchunk[k]
                ps = fps.pop(k)
                nc.tensor.matmul(ps[:], dgw[:], o2_t[b][:, r0:r0 + RC, :],
                                 start=False, stop=True)
                u = o2_t[b][:, r0:r0 + RC, :]
                if k % 2 == 0:
                    nc.scalar.activation(u, ps[:], AF.Relu, bias=b2c[:],
                                         scale=1.0)
                else:
                    nc.vector.tensor_scalar(
                        out=u, in0=ps[:], scalar1=b2c[:], scalar2=0.0,
                        op0=ALU.add, op1=ALU.max)
                eng = nc.sync if k % 2 == 0 else nc.scalar
                eng.dma_start(y_d.ap()[b][:, r0 * W:(r0 + RC) * W], u)
                if k + 8 < NFC:
                    prestage(k + 8)

    nc.compile()
    return nc


def _prep_inputs(x, w1, alpha1, gamma1, beta1, w2, alpha2, gamma2, beta2):
    import ml_dtypes
    bf = ml_dtypes.bfloat16
    x = np.asarray(x, dtype=np.float32)
    wq1, as1 = _quantize_int(np.asarray(w1), np.asarray(alpha1))
    wq2, as2 = _quantize_int(np.asarray(w2), np.asarray(alpha2))

    # [cout, cin, kh, kw] -> [cin, tap, cout] so lhsT slices are [K=cin, M=cout]
    w1t = np.ascontiguousarray(
        wq1.reshape(C, C, 9).transpose(1, 2, 0)).astype(bf)
    w2t = np.ascontiguousarray(
        wq2.reshape(C, C, 9).transpose(1, 2, 0)).astype(bf)

    ga1 = (np.asarray(gamma1, np.float32) * as1).reshape(C, 1)
    ga2 = (np.asarray(gamma2, np.float32) * as2).reshape(C, 1)
    be1 = np.asarray(beta1, np.float32).reshape(C, 1).copy()
    be2 = np.asarray(beta2, np.float32).reshape(C, 1).copy()
    ident = np.eye(C, dtype=bf)

    xpad = np.zeros((B, C, HP, WP), dtype=bf)
    xpad[:, :, 1:1 + H, 1:1 + W] = x.astype(bf)

    in_maps = []
    for c in range(N_CORES):
        shard = xpad[c * BL:(c + 1) * BL]
        in_maps.append({
            "xp": np.ascontiguousarray(shard),
            "w1t": w1t, "w2t": w2t, "ident": ident,
            "ga1": ga1, "be1": be1, "ga2": ga2, "be2": be2,
        })
    return in_maps, float(as1), float(as2)


def kernel(**inputs) -> np.ndarray:
    global LAST_RESULTS
    from concourse.bass_utils import run_bass_kernel_spmd

    in_maps, as1, as2 = _prep_inputs(**inputs)
    nc = _build_program(as1, as2)

    trace = bool(int(os.environ.get("KERNEL_TRACE", "0")))
    res = run_bass_kernel_spmd(
        nc, in_maps, list(range(N_CORES)),
        trace=trace,
    )
    LAST_RESULTS = res
    out = np.stack([np.asarray(res.results[c]["y"]) for c in range(N_CORES)])
    return np.ascontiguousarray(
        out.reshape(B, C, H, W)).astype(np.float32)
